# revision 1
# baseline (speedup 1.0000x reference)
"""AttnBlock (GroupNorm -> QKV -> full attention -> proj + residual) on 8
Trainium2 NeuronCores, data-parallel over the batch dimension (b=8, one
sample per core).

Layouts per core (sample):
  x:  (c=512, w=2048) fp32, channel tiles of 128 partitions.
  h:  GroupNorm(x) in f32r (feeds all matmuls; stays resident as the
  scores rhs).
  u = A.T h with A = (wq.T wk)/sqrt(c) folded on host (k never computed);
  scores_i = u[:, i-tile].T @ h; softmax without max-subtraction (scores
  are O(1) here); exp fused with row-sum via ACT accum_out; normalize on
  DVE; att transposed via PE transpose-mode. vp = (wp wv).T-projected v
  (folded on host), so out_h = sum_j vp.T @ attT needs no final proj;
  residual added from a streamed x slice, biases folded exactly (bk/bq
  cross-terms cancel in softmax or become a per-partition bias g on u).
"""

import functools

import numpy as np

B = 8
C = 512
W = 2048
G = 32
EPS = 1e-6
P = 128
CT = C // P          # 4 channel tiles
NW = W // 512        # 4 w-chunks of 512
IT = W // P          # 16 i-tiles
IGRP = 4             # i-tiles per ho/proj group
NG = IT // IGRP      # 4 groups

TRACE = False
DEBUG = False
LAST_EXEC_NS = None
LAST_TRACE_PATH = None


def _build_nc():
    import concourse.bass as bass
    import concourse.mybir as mybir
    import concourse.tile as tile
    from concourse import bacc
    from concourse.masks import make_identity

    f32 = mybir.dt.float32
    f32r = mybir.dt.float32r
    Ident = mybir.ActivationFunctionType.Identity
    Exp = mybir.ActivationFunctionType.Exp
    Sqrt = mybir.ActivationFunctionType.Sqrt
    mult = mybir.AluOpType.mult
    add = mybir.AluOpType.add
    subtract = mybir.AluOpType.subtract

    nc = bacc.Bacc()

    x_d = nc.declare_dram_parameter("x", [C, W], f32, isOutput=False)
    # Host-folded weights, partition-major [P, CT*C]:
    # A = (wq.T @ wk) * c^-0.5  (scores = h.T A h), WPV = (wp @ wv).T
    # (out_h = WPV.T h attT). k-projection and final proj are never computed.
    aT_d = nc.declare_dram_parameter("aT", [P, CT * C], f32, isOutput=False)
    wpvT_d = nc.declare_dram_parameter("wpvT", [P, CT * C], f32, isOutput=False)
    # One packed small-constant parameter (partition-major):
    # [0:512] per-tile group-avg selector S, [512:1024] selector-back ST,
    # then bq, bk, bp, gam, bet (CT cols each).
    aux_d = nc.declare_dram_parameter("aux", [P, 1044], f32, isOutput=False)
    out_d = nc.declare_dram_parameter("out", [C, W], f32, isOutput=True)

    with tile.TileContext(nc) as tc:
        with (
            tc.tile_pool(name="singles", bufs=1) as singles,
            tc.tile_pool(name="ps_small", bufs=8, space="PSUM") as ps_small,
            tc.tile_pool(name="qk", bufs=1) as qkp,
            tc.tile_pool(name="vt", bufs=1) as vtp,
            tc.tile_pool(name="gn", bufs=2) as gnp,
        ):
            # Pool nesting (LIFO): wqkv > hp > xp. x DMAs are emitted first
            # so they lead the sync queues; weight DMAs follow.
            wqkv_cm = tc.tile_pool(name="wqkv", bufs=1)
            wqkv = wqkv_cm.__enter__()
            a_sb = wqkv.tile([P, CT, C], f32r, name="a_sb")
            wpv_sb = wqkv.tile([P, CT, C], f32r, name="wpv_sb")
            a_sb_l = [a_sb[:, t, :] for t in range(CT)]
            wpv_sb_l = [wpv_sb[:, t, :] for t in range(CT)]
            h_sb = [qkp.tile([P, W], f32r, name=f"h{t}") for t in range(CT)]
            xp_cm = tc.tile_pool(name="xp", bufs=1)
            xp = xp_cm.__enter__()
            x_sb = [xp.tile([P, W], f32, name=f"x{t}") for t in range(CT)]

            # ---- singles (tiny DMAs first so they aren't queued behind x) ----
            ident = singles.tile([P, P], f32, name="ident")
            make_identity(nc, ident)
            ident_r = singles.tile([P, P], f32r, name="ident_r")
            nc.vector.tensor_copy(out=ident_r, in_=ident)
            eps_t = singles.tile([P, 1], f32, name="eps_t")
            nc.vector.memset(eps_t, EPS)
            aux_sb = singles.tile([P, 1044], f32, name="aux_sb")
            nc.sync.dma_start(out=aux_sb, in_=aux_d[:, :])
            s_sb = aux_sb[:, 0:512].rearrange("p (t g) -> p t g", t=CT)
            st_sb = aux_sb[:, 512:1024].rearrange("p (t c) -> p t c", t=CT)
            g_sb = aux_sb[:, 1024:1028]
            bp_sb = aux_sb[:, 1032:1036]
            gam_sb = aux_sb[:, 1036:1040]
            bet_sb = aux_sb[:, 1040:1044]
            nc.sync.dma_start(out=x_sb[0], in_=x_d[0 * P:1 * P, :])
            nc.sync.dma_start(out=x_sb[1], in_=x_d[1 * P:2 * P, :])
            nc.sync.dma_start(out=a_sb, in_=aT_d[:, :].bitcast(f32r))
            nc.sync.dma_start(out=x_sb[2], in_=x_d[2 * P:3 * P, :])
            for hw in range(2):
                nc.sync.dma_start(
                    out=x_sb[3][:, hw * 1024:(hw + 1) * 1024],
                    in_=x_d[3 * P:4 * P, hw * 1024:(hw + 1) * 1024])
            nc.sync.dma_start(out=wpv_sb, in_=wpvT_d[:, :].bitcast(f32r))

            if True:
                pass
                # ===== GroupNorm: stats pass for all tiles first (keeps
                # DVE free of head-of-line blocking on the per-tile chains)
                st2_l = []
                for t in range(CT):
                    stats = gnp.tile([P, NW, 6], f32, tag="bnstats", name=f"bns{t}")
                    for sg in range(NW):
                        nc.vector.bn_stats(out=stats[:, sg, :],
                                           in_=x_sb[t][:, sg * 512:(sg + 1) * 512])
                    mv = gnp.tile([P, 2], f32, tag="mv", name=f"mv{t}")
                    nc.vector.bn_aggr(out=mv, in_=stats)
                    st2 = gnp.tile([P, 2], f32, tag=f"st2_{t}", name=f"st2_{t}")
                    nc.vector.tensor_copy(out=st2[:, 0:1], in_=mv[:, 0:1])
                    nc.vector.tensor_tensor(out=st2[:, 1:2], in0=mv[:, 0:1],
                                            in1=mv[:, 0:1], op=mult)
                    nc.vector.tensor_add(out=st2[:, 1:2], in0=st2[:, 1:2],
                                         in1=mv[:, 1:2])
                    st2_l.append(st2)
                def emit_gn_chain(t):
                    st2 = st2_l[t]
                    ps_g = ps_small.tile([P, 2], f32, tag="ps512", name=f"ps_g{t}")
                    nc.tensor.matmul(ps_g[:], lhsT=s_sb[:, t, :], rhs=st2,
                                     start=True, stop=True)
                    gsr = gnp.tile([P, 2], f32, tag="gsr", name=f"gsr{t}")
                    nc.vector.tensor_copy(out=gsr[:8, :], in_=ps_g[:8, :])
                    gs2 = gnp.tile([P, 2], f32, tag="gs2", name=f"gs2_{t}")
                    nc.vector.memset(gs2, 0.0)
                    nc.vector.tensor_copy(out=gs2[:8, 0:1], in_=gsr[:8, 0:1])
                    nc.vector.tensor_tensor(out=gs2[:8, 1:2], in0=gsr[:8, 0:1],
                                            in1=gsr[:8, 0:1], op=mult)
                    nc.vector.tensor_tensor(out=gs2[:8, 1:2], in0=gsr[:8, 1:2],
                                            in1=gs2[:8, 1:2], op=subtract)
                    nc.scalar.activation(out=gs2[:8, 1:2], in_=gs2[:8, 1:2],
                                         func=Sqrt, bias=eps_t[:8], scale=1.0)
                    nc.vector.reciprocal(gs2[:8, 1:2], gs2[:8, 1:2])
                    ps_bc = ps_small.tile([P, 2], f32, tag="ps512", name=f"psbc{t}")
                    nc.tensor.matmul(ps_bc[:], lhsT=st_sb[:, t, :],
                                     rhs=gs2, start=True, stop=True)
                    bca = gnp.tile([P, 2], f32, tag="bca", name=f"bca{t}")
                    nc.vector.tensor_copy(out=bca, in_=ps_bc)
                    alph = gnp.tile([P, 1], f32, tag=f"alph{t}", name=f"alph{t}")
                    nc.vector.tensor_tensor(out=alph, in0=bca[:, 1:2],
                                            in1=gam_sb[:, t:t + 1], op=mult)
                    beta = gnp.tile([P, 1], f32, tag=f"beta{t}", name=f"beta{t}")
                    nc.vector.tensor_tensor(out=beta, in0=bca[:, 0:1],
                                            in1=alph, op=mult)
                    nc.vector.tensor_tensor(out=beta, in0=bet_sb[:, t:t + 1],
                                            in1=beta, op=subtract)
                    if t % 2 == 0:
                        nc.scalar.activation(out=h_sb[t], in_=x_sb[t],
                                             func=Ident, scale=alph, bias=beta)
                    else:
                        nc.vector.tensor_scalar(out=h_sb[t], in0=x_sb[t],
                                                scalar1=alph, scalar2=beta,
                                                op0=mult, op1=add)

                # ================= u = A.T h  and  vp = WPV.T h =========
                u_sb = [qkp.tile([P, W], f32r, name=f"u{t}") for t in range(CT)]
                vp_sb = [vtp.tile([P, C], f32r, name=f"vp{j}") for j in range(IT)]

                def emit_phase(grp, pss, ct):
                    for ch in grp:
                        kind, a, b = ch
                        if kind == "u":
                            lhsT = a_sb_l[ct][:, a * P:(a + 1) * P]
                            rhs = h_sb[ct][:, b * 512:(b + 1) * 512]
                        else:
                            lhsT = h_sb[ct][:, a * P:(a + 1) * P]
                            rhs = wpv_sb_l[ct]
                        nc.tensor.matmul(pss[ch][:], lhsT=lhsT, rhs=rhs,
                                         start=(ct == 0), stop=(ct == CT - 1))

                # First 6 u-chains phase-woven with the GN tile chains.
                grp0 = ([("u", 0, jc) for jc in range(NW)]
                        + [("u", 1, 0), ("u", 1, 1)])
                pss0 = {}
                for ch in grp0:
                    pss0[ch] = ps_small.tile([P, 512], f32, tag="ps512",
                                             name=f"psu0_{ch[1]}_{ch[2]}")
                for t in range(CT):
                    emit_gn_chain(t)
                    emit_phase(grp0, pss0, t)
                xp_cm.__exit__(None, None, None)
                for ch in grp0:
                    _, a, b = ch
                    nc.scalar.activation(
                        out=u_sb[a][:, b * 512:(b + 1) * 512],
                        in_=pss0[ch], func=Ident,
                        bias=g_sb[:, a:a + 1], scale=1.0)

                chains = ([("u", 1, 2), ("u", 1, 3)]
                          + [("u", ot, jc) for ot in range(2, CT) for jc in range(NW)]
                          + [("v", jt, 0) for jt in range(IT)])
                for g8 in range(0, len(chains), 8):
                    grp = chains[g8:g8 + 8]
                    pss = {}
                    for ch in grp:
                        pss[ch] = ps_small.tile(
                            [P, 512], f32, tag="ps512",
                            name=f"psqkv{ch[0]}{ch[1]}_{ch[2]}")
                    for ct in range(CT):
                        emit_phase(grp, pss, ct)
                    for ch in grp:
                        kind, a, b = ch
                        if kind == "u":
                            nc.scalar.activation(
                                out=u_sb[a][:, b * 512:(b + 1) * 512],
                                in_=pss[ch], func=Ident,
                                bias=g_sb[:, a:a + 1], scale=1.0)
                        else:
                            nc.vector.tensor_copy(out=vp_sb[a], in_=pss[ch])

            wqkv_cm.__exit__(None, None, None)

            # ================= Attention + proj =================
            attn_pools = (
                tc.tile_pool(name="attT", bufs=1),
                tc.tile_pool(name="att", bufs=3),
                tc.tile_pool(name="outp", bufs=2),
                tc.tile_pool(name="xs", bufs=2),
            )
            attTp = attn_pools[0].__enter__()
            attp = attn_pools[1].__enter__()
            outp = attn_pools[2].__enter__()
            xsp = attn_pools[3].__enter__()
            att_tiles = {}
            attT_by_g = {}

            def get_attT(g):
                if g not in attT_by_g:
                    attT_by_g[g] = attTp.tile([P, IT, 512], f32r, tag="attT",
                                              name=f"attT{g}")
                return attT_by_g[g]

            def emit_scores(it):
                att = attp.tile([P, W], f32r, tag="att", name=f"att{it}")
                att_tiles[it] = att
                srows = gnp.tile([P, NW], f32, tag="srows", name=f"srows{it}")
                for jc in range(NW):
                    ps_s = ps_small.tile([P, 512], f32, tag="ps512",
                                         name=f"sc{it}_{jc}")
                    for ct in range(CT):
                        nc.tensor.matmul(
                            ps_s[:],
                            lhsT=u_sb[ct][:, it * P:(it + 1) * P],
                            rhs=h_sb[ct][:, jc * 512:(jc + 1) * 512],
                            start=(ct == 0), stop=(ct == CT - 1))
                    nc.scalar.activation(out=att[:, jc * 512:(jc + 1) * 512],
                                         in_=ps_s, func=Exp,
                                         bias=0.0, scale=1.0,
                                         accum_out=srows[:, jc:jc + 1])
                srow = gnp.tile([P, 1], f32, tag="srow", name=f"srow{it}")
                nc.vector.reduce_sum(srow, srows, axis=mybir.AxisListType.X)
                rec = gnp.tile([P, 1], f32, tag="rec", name=f"rec{it}")
                nc.vector.reciprocal(rec, srow)
                nc.vector.tensor_scalar_mul(att, att, rec)

            def emit_transposes(it):
                att = att_tiles.pop(it)
                s = it % IGRP
                attT = get_attT(it // IGRP)
                for jt4 in range(4):
                    ps_t = ps_small.tile([P, 512], f32r, tag="ps512",
                                         name=f"pst{it}_{jt4}")
                    for j4 in range(4):
                        jt = jt4 * 4 + j4
                        nc.tensor.transpose(
                            ps_t[:, j4 * P:(j4 + 1) * P],
                            att[:, jt * P:(jt + 1) * P], ident_r)
                    nc.vector.tensor_copy(
                        out=attT[:, jt4 * 4:jt4 * 4 + 4, s * P:(s + 1) * P],
                        in_=ps_t.rearrange("p (a b) -> p a b", a=4))

            def emit_ho_proj(g):
                attT = attT_by_g.pop(g)
                for ot in range(CT):
                    ps_o = ps_small.tile([P, 512], f32, tag="ps512",
                                         name=f"pso{g}_{ot}")
                    for jt in range(IT):
                        nc.tensor.matmul(
                            ps_o[:],
                            lhsT=vp_sb[jt][:, ot * P:(ot + 1) * P],
                            rhs=attT[:, jt, :],
                            start=(jt == 0), stop=(jt == IT - 1))
                    xs = xsp.tile([P, 512], f32, tag="xs", name=f"xs{g}_{ot}")
                    nc.sync.dma_start(
                        out=xs,
                        in_=x_d[ot * P:(ot + 1) * P, g * 512:(g + 1) * 512])
                    tmp = outp.tile([P, 512], f32, tag="tmp", name=f"tmp{g}_{ot}")
                    nc.vector.tensor_add(out=tmp, in0=ps_o, in1=xs)
                    osb = outp.tile([P, 512], f32, tag="osb", name=f"osb{g}_{ot}")
                    nc.scalar.activation(out=osb, in_=tmp, func=Ident,
                                         bias=bp_sb[:, ot:ot + 1], scale=1.0)
                    nc.sync.dma_start(
                        out=out_d[ot * P:(ot + 1) * P, g * 512:(g + 1) * 512],
                        in_=osb)

            for step in range(IT + 1):
                if step < IT:
                    emit_scores(step)
                if step >= 1:
                    emit_transposes(step - 1)
                if step % IGRP == 0 and step >= IGRP:
                    emit_ho_proj(step // IGRP - 1)
            for pcm in reversed(attn_pools):
                pcm.__exit__(None, None, None)

    nc.finalize()
    return nc


@functools.lru_cache(maxsize=1)
def _built():
    return _build_nc()


def kernel(x, gn_gamma, gn_beta, wq, bq, wk, bk, wv, bv, wp, bp):
    global LAST_EXEC_NS, LAST_TRACE_PATH
    from concourse.bass_utils import run_bass_kernel_spmd

    x = np.asarray(x, dtype=np.float32)
    scale = float(C) ** -0.5
    f = np.float32
    def pmajor(wT):
        # (C_in, C_out) -> [P, CT*C]: row p holds tiles t=0..CT-1 of wT
        return np.ascontiguousarray(
            wT.reshape(CT, P, C).transpose(1, 0, 2).reshape(P, CT * C))

    f64 = np.float64
    wq64 = np.asarray(wq, f64)
    wk64 = np.asarray(wk, f64)
    wv64 = np.asarray(wv, f64)
    wp64 = np.asarray(wp, f64)
    # scores = h.T A h + (wk.T bq~).h  (bk terms are per-row constants that
    # cancel in softmax); out_h = (wp wv h) attT
    aT = pmajor((wq64.T @ wk64 * scale).astype(f))
    wpvT = pmajor((wp64 @ wv64).T.astype(f))
    g_vec = (wk64.T @ (np.asarray(bq, f64) * scale)).astype(f)
    # v and out biases fold through the row-stochastic attention into bp
    bp_eff = (np.asarray(bp, f64) + wp64 @ np.asarray(bv, f64)).astype(f).reshape(C, 1)
    gam = np.asarray(gn_gamma, f).reshape(C, 1)
    bet = np.asarray(gn_beta, f).reshape(C, 1)

    gsz = C // G
    aux = np.zeros((P, 1044), dtype=f)
    for t in range(CT):
        for p in range(P):
            aux[p, t * P + p // gsz] = 1.0 / gsz          # S selector
            for cl in range(P):
                if p == cl // gsz:
                    aux[p, 512 + t * P + cl] = 1.0        # ST selector
    aux[:, 1024:1028] = g_vec.reshape(CT, P).T
    aux[:, 1032:1036] = bp_eff.reshape(CT, P).T
    aux[:, 1036:1040] = gam.reshape(CT, P).T
    aux[:, 1040:1044] = bet.reshape(CT, P).T

    shared = dict(aT=aT, wpvT=wpvT, aux=aux)
    in_maps = [dict(x=np.ascontiguousarray(x[i]), **shared) for i in range(B)]

    nc = _built()
    last_err = None
    for attempt in range(3):
        try:
            res = run_bass_kernel_spmd(nc, in_maps, list(range(B)), trace=TRACE)
            out = np.stack([np.asarray(res.results[i]["out"], dtype=np.float32)
                            for i in range(B)], axis=0)
            break
        except Exception as e:  # transient NRT device errors: retry
            last_err = e
            if attempt == 2:
                raise
            import time
            time.sleep(2.0)
    if TRACE:
        LAST_EXEC_NS = res.exec_time_ns
        if res.instructions_and_trace is not None:
            LAST_TRACE_PATH = res.instructions_and_trace[1]
    return out



# revision 3
# speedup vs baseline: 1.4887x; 1.4887x over previous
"""AttnBlock (GroupNorm -> QKV -> full attention -> proj + residual) on 8
Trainium2 NeuronCores, data-parallel over batch (b=8, one sample per core).

fp8 (e4m3, max 240) DoubleRow pipeline. Per core:
  h = GroupNorm(x) written directly as fp8 pairs (scale Sh folded into
  gamma/beta). Weights folded on host: A = (wq.T wk)/sqrt(c) (k/q never
  computed), WPV = (wp wv). u = A.T h + g. scoresT[j,i] = h.T u computed
  DIRECTLY transposed (no PE transposes, no PSUM->SBUF copies): exp
  activation writes attT fp8 straight to SBUF with a global shift (exact
  softmax identity). Row sums via ones-DoubleRow matmul [32,512];
  transposed to per-partition layout by tiny PE transposes. outT[i,c] =
  attT.T @ vp computed i-major so 1/rowsum is a per-partition activation
  scale. Final PE transpose back to c-major (bf16) + fused
  (psum+bp)+x residual on DVE. All big matmuls fp8 DoubleRow (2 K-tiles
  per instruction).
"""

import functools

import numpy as np

B = 8
C = 512
W = 2048
G = 32
EPS = 1e-6
P = 128
CT = C // P          # 4 channel tiles
CP = CT // 2         # 2 channel-tile pairs
NW = W // 512        # 4 w-chunks of 512
IT = W // P          # 16 i-tiles
JP = IT // 2         # 8 j-tile pairs

SA = 256.0           # fp8 scale for A
SH = 16.0            # fp8 scale for h
SU = 8.0             # fp8 scale for u
SVP = 4.0            # fp8 scale for vp
SWPV = 64.0          # fp8 scale for WPV weights
ASHIFT = 1.5         # global score shift before exp (cancels in softmax)

TRACE = False
LAST_EXEC_NS = None
LAST_TRACE_PATH = None


def _build_nc():
    import concourse.mybir as mybir
    import concourse.tile as tile
    from concourse import bacc
    from concourse.masks import make_identity

    f32 = mybir.dt.float32
    f8 = mybir.dt.float8e4
    bf16 = mybir.dt.bfloat16
    Ident = mybir.ActivationFunctionType.Identity
    Exp = mybir.ActivationFunctionType.Exp
    Sqrt = mybir.ActivationFunctionType.Sqrt
    mult = mybir.AluOpType.mult
    add = mybir.AluOpType.add
    subtract = mybir.AluOpType.subtract
    DR = mybir.MatmulPerfMode.DoubleRow

    nc = bacc.Bacc()

    x_d = nc.declare_dram_parameter("x", [C, W], f32, isOutput=False)
    # fp8 weights, pair-major [P, CP*2*C]: [p, cp, s, co] = w[(2cp+s)*128+p, co]
    a8_d = nc.declare_dram_parameter("a8", [P, CP * 2 * C], f8, isOutput=False)
    wpv8_d = nc.declare_dram_parameter("wpv8", [P, CP * 2 * C], f8, isOutput=False)
    # packed constants: [0:512] group-avg selector S, [512:1024] selector-back
    # ST, then gSu, bp_eff, gam*SH, bet*SH (CT cols each).
    aux_d = nc.declare_dram_parameter("aux", [P, 1040], f32, isOutput=False)
    out_d = nc.declare_dram_parameter("out", [C, W], f32, isOutput=True)

    with tile.TileContext(nc) as tc:
        with (
            tc.tile_pool(name="singles", bufs=1) as singles,
            tc.tile_pool(name="w8", bufs=1) as w8p,
            tc.tile_pool(name="xp", bufs=1) as xp,
            tc.tile_pool(name="h8p", bufs=1) as h8p,
            tc.tile_pool(name="att8p", bufs=1) as att8p,
            tc.tile_pool(name="outp", bufs=1) as outp,
            tc.tile_pool(name="gn", bufs=2) as gnp,
        ):
            # ---- persistent SBUF ----
            a8_sb = w8p.tile([P, CP, 2, C], f8, name="a8_sb")
            wpv8_sb = w8p.tile([P, CP, 2, C], f8, name="wpv8_sb")
            x_sb = [xp.tile([P, W], f32, name=f"x{t}") for t in range(CT)]
            h8 = h8p.tile([P, CP, 2, W], f8, name="h8")
            u8 = h8p.tile([P, CP, 2, W], f8, name="u8")
            vp8 = [h8p.tile([P, 2, C], f8, name=f"vp8_{jp}") for jp in range(JP)]
            att8 = [att8p.tile([P, 2, W], f8, name=f"att8_{jp}") for jp in range(JP)]
            outT = [outp.tile([P, 512], bf16, name=f"outT{it}") for it in range(IT)]

            ident = singles.tile([P, P], f32, name="ident")
            make_identity(nc, ident)
            ident_b = singles.tile([P, P], bf16, name="ident_b")
            nc.vector.tensor_copy(out=ident_b, in_=ident)
            eps_t = singles.tile([P, 1], f32, name="eps_t")
            nc.vector.memset(eps_t, EPS)
            expb_t = singles.tile([P, 1], f32, name="expb_t")
            nc.vector.memset(expb_t, -ASHIFT)
            ones8 = singles.tile([P, 2, 32], f8, name="ones8")
            nc.vector.memset(ones8, 1.0)
            rs_sb = singles.tile([1, W], f32, name="rs_sb")
            rec_sb = singles.tile([P, IT], f32, name="rec_sb")
            aux_sb = singles.tile([P, 1040], f32, name="aux_sb")
            nc.sync.dma_start(out=aux_sb, in_=aux_d[:, :])
            s_sb = aux_sb[:, 0:512].rearrange("p (t g) -> p t g", t=CT)
            st_sb = aux_sb[:, 512:1024].rearrange("p (t c) -> p t c", t=CT)
            gsu_sb = aux_sb[:, 1024:1028]
            bp_sb = aux_sb[:, 1028:1032]
            gam_sb = aux_sb[:, 1032:1036]
            bet_sb = aux_sb[:, 1036:1040]

            nc.sync.dma_start(out=x_sb[0], in_=x_d[0 * P:1 * P, :])
            nc.sync.dma_start(out=x_sb[1], in_=x_d[1 * P:2 * P, :])
            nc.sync.dma_start(
                out=a8_sb, in_=a8_d[:, :].rearrange("p (c s o) -> p c s o", c=CP, s=2))
            nc.sync.dma_start(out=x_sb[2], in_=x_d[2 * P:3 * P, :])
            nc.sync.dma_start(
                out=wpv8_sb,
                in_=wpv8_d[:, :].rearrange("p (c s o) -> p c s o", c=CP, s=2))
            for hw in range(2):
                nc.sync.dma_start(
                    out=x_sb[3][:, hw * 1024:(hw + 1) * 1024],
                    in_=x_d[3 * P:4 * P, hw * 1024:(hw + 1) * 1024])

            ps_a_cm = tc.tile_pool(name="ps_a", bufs=8, space="PSUM")
            ps_a = ps_a_cm.__enter__()

            # ===== GroupNorm: stats for all tiles, then per-tile chains =====
            st2_l = []
            for t in range(CT):
                stats = gnp.tile([P, NW, 6], f32, tag="bnstats", name=f"bns{t}")
                for sg in range(NW):
                    nc.vector.bn_stats(out=stats[:, sg, :],
                                       in_=x_sb[t][:, sg * 512:(sg + 1) * 512])
                mv = gnp.tile([P, 2], f32, tag="mv", name=f"mv{t}")
                nc.vector.bn_aggr(out=mv, in_=stats)
                st2 = gnp.tile([P, 2], f32, tag=f"st2_{t}", name=f"st2_{t}")
                nc.vector.tensor_copy(out=st2[:, 0:1], in_=mv[:, 0:1])
                nc.vector.tensor_tensor(out=st2[:, 1:2], in0=mv[:, 0:1],
                                        in1=mv[:, 0:1], op=mult)
                nc.vector.tensor_add(out=st2[:, 1:2], in0=st2[:, 1:2],
                                     in1=mv[:, 1:2])
                st2_l.append(st2)
            for t in range(CT):
                st2 = st2_l[t]
                ps_g = ps_a.tile([P, 2], f32, tag="ps512", name=f"ps_g{t}")
                nc.tensor.matmul(ps_g[:], lhsT=s_sb[:, t, :], rhs=st2,
                                 start=True, stop=True)
                gsr = gnp.tile([P, 2], f32, tag="gsr", name=f"gsr{t}")
                nc.vector.tensor_copy(out=gsr[:8, :], in_=ps_g[:8, :])
                gs2 = gnp.tile([P, 2], f32, tag="gs2", name=f"gs2_{t}")
                nc.vector.memset(gs2, 0.0)
                nc.vector.tensor_copy(out=gs2[:8, 0:1], in_=gsr[:8, 0:1])
                nc.vector.tensor_tensor(out=gs2[:8, 1:2], in0=gsr[:8, 0:1],
                                        in1=gsr[:8, 0:1], op=mult)
                nc.vector.tensor_tensor(out=gs2[:8, 1:2], in0=gsr[:8, 1:2],
                                        in1=gs2[:8, 1:2], op=subtract)
                nc.scalar.activation(out=gs2[:8, 1:2], in_=gs2[:8, 1:2],
                                     func=Sqrt, bias=eps_t[:8], scale=1.0)
                nc.vector.reciprocal(gs2[:8, 1:2], gs2[:8, 1:2])
                ps_bc = ps_a.tile([P, 2], f32, tag="ps512", name=f"psbc{t}")
                nc.tensor.matmul(ps_bc[:], lhsT=st_sb[:, t, :],
                                 rhs=gs2, start=True, stop=True)
                bca = gnp.tile([P, 2], f32, tag="bca", name=f"bca{t}")
                nc.vector.tensor_copy(out=bca, in_=ps_bc)
                alph = gnp.tile([P, 1], f32, tag=f"alph{t}", name=f"alph{t}")
                nc.vector.tensor_tensor(out=alph, in0=bca[:, 1:2],
                                        in1=gam_sb[:, t:t + 1], op=mult)
                beta = gnp.tile([P, 1], f32, tag=f"beta{t}", name=f"beta{t}")
                nc.vector.tensor_tensor(out=beta, in0=bca[:, 0:1],
                                        in1=alph, op=mult)
                nc.vector.tensor_tensor(out=beta, in0=bet_sb[:, t:t + 1],
                                        in1=beta, op=subtract)
                h8_sl = h8[:, t // 2, t % 2, :]
                if t % 2 == 0:
                    nc.scalar.activation(out=h8_sl, in_=x_sb[t],
                                         func=Ident, scale=alph, bias=beta)
                else:
                    nc.vector.tensor_scalar(out=h8_sl, in0=x_sb[t],
                                            scalar1=alph, scalar2=beta,
                                            op0=mult, op1=add)

            # ===== u8 = fp8(Su*(A.T h + g)); vp8 = fp8(Svp * WPV.T h) =====
            for jc in range(NW):
                for co in range(CT):
                    ps_u = ps_a.tile([P, 512], f32, tag="ps512",
                                     name=f"psu{jc}_{co}")
                    for cp in range(CP):
                        nc.tensor.matmul(
                            ps_u[:],
                            lhsT=a8_sb[:, cp, :, co * P:(co + 1) * P],
                            rhs=h8[:, cp, :, jc * 512:(jc + 1) * 512],
                            start=(cp == 0), stop=(cp == CP - 1), perf_mode=DR)
                    nc.scalar.activation(
                        out=u8[:, co // 2, co % 2, jc * 512:(jc + 1) * 512],
                        in_=ps_u, func=Ident, scale=SU / (SA * SH),
                        bias=gsu_sb[:, co:co + 1])
            for jt in range(IT):
                ps_v = ps_a.tile([P, 512], f32, tag="ps512", name=f"psv{jt}")
                for cp in range(CP):
                    nc.tensor.matmul(
                        ps_v[:],
                        lhsT=h8[:, cp, :, jt * P:(jt + 1) * P],
                        rhs=wpv8_sb[:, cp, :, :],
                        start=(cp == 0), stop=(cp == CP - 1), perf_mode=DR)
                nc.vector.tensor_scalar_mul(vp8[jt // 2][:, jt % 2, :], ps_v,
                                            SVP / (SH * SWPV))
            ps_a_cm.__exit__(None, None, None)

            # ===== scoresT + exp -> attT fp8 (j-major, no transposes) =====
            sc_cm = tc.tile_pool(name="ps_sc", bufs=2, space="PSUM")
            ps_sc = sc_cm.__enter__()
            for jt in range(IT):
                sc = ps_sc.tile([P, NW, 512], f32, tag="sc", name=f"sc{jt}")
                for jc in range(NW):
                    for cp in range(CP):
                        nc.tensor.matmul(
                            sc[:, jc, :],
                            lhsT=h8[:, cp, :, jt * P:(jt + 1) * P],
                            rhs=u8[:, cp, :, jc * 512:(jc + 1) * 512],
                            start=(cp == 0), stop=(cp == CP - 1), perf_mode=DR)
                nc.scalar.activation(out=att8[jt // 2][:, jt % 2, :], in_=sc,
                                     func=Exp, scale=1.0 / (SH * SU),
                                     bias=expb_t)
            sc_cm.__exit__(None, None, None)

            # ===== row sums -> rec (per-partition for i-major normalize) ====
            ps_d_cm = tc.tile_pool(name="ps_d", bufs=1, space="PSUM")
            ps_d = ps_d_cm.__enter__()
            for g in range(NW):
                ps_r = ps_d.tile([32, 512], f32, tag="rs", bufs=1,
                                 name=f"ps_r{g}")
                for jp in range(JP):
                    nc.tensor.matmul(
                        ps_r[:], lhsT=ones8[:, :, :],
                        rhs=att8[jp][:, :, g * 512:(g + 1) * 512],
                        start=(jp == 0), stop=(jp == JP - 1), perf_mode=DR)
                nc.scalar.activation(out=rs_sb[0:1, g * 512:(g + 1) * 512],
                                     in_=ps_r[0:1, :], func=Ident,
                                     scale=SVP, bias=0.0)
            ps_rt = ps_d.tile([P, IT], f32, tag="rst", bufs=1, name="ps_rt")
            for it in range(IT):
                nc.tensor.transpose(ps_rt[:, it:it + 1],
                                    rs_sb[0:1, it * P:(it + 1) * P],
                                    ident[0:1, 0:1])
            nc.vector.reciprocal(rec_sb, ps_rt)

            # ===== outT = attT.T @ vp (i-major); normalize; transpose back ==
            def emit_outT(it):
                ps_o = ps_d.tile([P, 512], f32, tag="o", bufs=4,
                                 name=f"ps_o{it}")
                for jp in range(JP):
                    nc.tensor.matmul(
                        ps_o[:],
                        lhsT=att8[jp][:, :, it * P:(it + 1) * P],
                        rhs=vp8[jp][:, :, :],
                        start=(jp == 0), stop=(jp == JP - 1), perf_mode=DR)
                nc.scalar.activation(out=outT[it], in_=ps_o, func=Ident,
                                     scale=rec_sb[:, it:it + 1], bias=0.0)

            def emit_final(g):
                for ot in range(CT):
                    ps_t = ps_d.tile([P, 512], bf16, tag="tr", bufs=2,
                                     name=f"ps_t{g}_{ot}")
                    for k in range(4):
                        nc.tensor.transpose(
                            ps_t[:, k * P:(k + 1) * P],
                            outT[4 * g + k][:, ot * P:(ot + 1) * P], ident_b)
                    osb = outp.tile([P, 512], f32, tag="osb", bufs=4,
                                    name=f"osb{g}_{ot}")
                    nc.vector.scalar_tensor_tensor(
                        out=osb, in0=ps_t, scalar=bp_sb[:, ot:ot + 1],
                        in1=x_sb[ot][:, g * 512:(g + 1) * 512],
                        op0=add, op1=add)
                    nc.sync.dma_start(
                        out=out_d[ot * P:(ot + 1) * P, g * 512:(g + 1) * 512],
                        in_=osb)

            for w in range(NW + 1):
                if w < NW:
                    for it in range(4 * w, 4 * w + 4):
                        emit_outT(it)
                if w >= 1:
                    emit_final(w - 1)
            ps_d_cm.__exit__(None, None, None)

    nc.finalize()
    return nc


@functools.lru_cache(maxsize=1)
def _built():
    return _build_nc()


def _pair_major(wT):
    # (C_in, C_out) -> [P, CP*2*C]: [p, cp, s, co] = wT[(2cp+s)*128+p, co]
    return np.ascontiguousarray(
        wT.reshape(CP, 2, P, C).transpose(2, 0, 1, 3).reshape(P, CP * 2 * C))


def kernel(x, gn_gamma, gn_beta, wq, bq, wk, bk, wv, bv, wp, bp):
    global LAST_EXEC_NS, LAST_TRACE_PATH
    import ml_dtypes
    from concourse.bass_utils import run_bass_kernel_spmd

    E4 = ml_dtypes.float8_e4m3
    x = np.asarray(x, dtype=np.float32)
    scale = float(C) ** -0.5
    f = np.float32
    f64 = np.float64
    wq64 = np.asarray(wq, f64)
    wk64 = np.asarray(wk, f64)
    wv64 = np.asarray(wv, f64)
    wp64 = np.asarray(wp, f64)
    # scores = h.T A h + (wk.T bq scale).h; bk terms cancel in softmax.
    # out = (wp wv h) attT; bv/bp fold through row-stochastic att into bp.
    aT = (wq64.T @ wk64 * scale).astype(f)
    wpvT = (wp64 @ wv64).T.astype(f)
    a8 = _pair_major((aT * SA).astype(E4))
    wpv8 = _pair_major((wpvT * SWPV).astype(E4))
    g_vec = (wk64.T @ (np.asarray(bq, f64) * scale)).astype(f)
    bp_eff = (np.asarray(bp, f64) + wp64 @ np.asarray(bv, f64)).astype(f)
    gam = (np.asarray(gn_gamma, f) * SH).reshape(C, 1)
    bet = (np.asarray(gn_beta, f) * SH).reshape(C, 1)

    gsz = C // G
    aux = np.zeros((P, 1040), dtype=f)
    for t in range(CT):
        for p in range(P):
            aux[p, t * P + p // gsz] = 1.0 / gsz          # S selector
            for cl in range(P):
                if p == cl // gsz:
                    aux[p, 512 + t * P + cl] = 1.0        # ST selector
    aux[:, 1024:1028] = (g_vec * SU).reshape(CT, P).T
    aux[:, 1028:1032] = bp_eff.reshape(CT, P).T
    aux[:, 1032:1036] = gam.reshape(CT, P).T
    aux[:, 1036:1040] = bet.reshape(CT, P).T

    shared = dict(a8=a8, wpv8=wpv8, aux=aux)
    in_maps = [dict(x=np.ascontiguousarray(x[i]), **shared) for i in range(B)]

    nc = _built()
    last_err = None
    for attempt in range(3):
        try:
            res = run_bass_kernel_spmd(nc, in_maps, list(range(B)), trace=TRACE)
            out = np.stack([np.asarray(res.results[i]["out"], dtype=np.float32)
                            for i in range(B)], axis=0)
            break
        except Exception as e:  # transient NRT device errors: retry
            last_err = e
            if attempt == 2:
                raise
            import time
            time.sleep(2.0)
    if TRACE:
        LAST_EXEC_NS = res.exec_time_ns
        if res.instructions_and_trace is not None:
            LAST_TRACE_PATH = res.instructions_and_trace[1]
    return out
